# revision 13
# baseline (speedup 1.0000x reference)
"""DSAttention (de-stationary causal attention) Trainium2 Bass kernel.

Problem: B=4, L=S=2048, H=8, E=D=64, f32.
  scores = (Q @ K^T) * tau[b] + delta[b, j]
  A = softmax(scale * scores + causal mask), scale = 1/sqrt(E)
  out = A @ V
Sharding: B*H = 32 independent (b,h) heads -> 4 per core on 8 cores.

Device algorithm per (b,h), S^T formulation (j on partitions) so the softmax
denominator falls out of the PV matmul:
  - Host pre-transposes to bf16: qt[e, i] = 0.125*tau[b]*Q[i, e],
    kt[e, j] = K[j, e] (K=64 contraction).  The de-stationary bias
    0.125*delta[b, j] is applied on the ACT engine's per-partition bias port
    during the exp, so no extra contraction row is needed.
  - For each i-half (IH=1024 PSUM window) and j-chunk (JC=128 rows):
      ps[j, i] = kt_c.T @ qt[:, x:1024]          one bf16 matmul (<=1024 wide)
      et       = exp(ps + delta_j)  (ACT, PSUM f32 -> SBUF bf16, bias port)
      diagonal chunk: et[:, 0:128] *= triu mask  (DVE, bf16 4x mode)
      po[65, i] += vp_c.T @ et                   accumulating bf16 matmul
    vp carries a ones column so po row 64 is the softmax denominator.
  - po -> SBUF f32 (DVE) -> HBM.  Host divides by row 64 and transposes.
bf16 moving operands allow 1024-wide matmuls (fp32 caps at 512) and FWL
fast weight loads; accuracy margin vs the 2e-2 gate is ~4x (validated).
"""

import numpy as np
import ml_dtypes

B, L, SEQ, H, E, D = 4, 2048, 2048, 8, 64, 64
N_CORES = 8
PAIRS = (B * H) // N_CORES  # 4 (b,h) pairs per core
SCALE = 1.0 / float(np.sqrt(E))  # 0.125
JC = 128               # j-chunk (key rows per tile, PSUM partition dim)
IH = 1024              # i-half width (PSUM free dim budget)
N_CHUNKS = SEQ // JC   # 16

_CACHED = {}


def _build_bass():
    key = "nc"
    if key in _CACHED:
        return _CACHED[key]
    import concourse.mybir as mybir
    import concourse.tile as tile
    from concourse import bacc

    f32 = mybir.dt.float32
    bf16 = mybir.dt.bfloat16
    EXP = mybir.ActivationFunctionType.Exp

    nc = bacc.Bacc("TRN2", target_bir_lowering=False, debug=False)

    # qk0 = the first chunk's operands (kt chunk 0 + qt bank 0) so the PE can
    # start after 160 KB; qkA = rest of the first i/j halves (qt[:, :1024] |
    # kt[:, :1024]); qkB = second halves.
    qk0 = nc.dram_tensor("qk0", [PAIRS, JC, 512 + JC], bf16,
                         kind="ExternalInput").ap()
    qkA = nc.dram_tensor("qkA", [PAIRS, JC, 2 * IH], bf16,
                         kind="ExternalInput").ap()
    qkB = nc.dram_tensor("qkB", [PAIRS, JC, 2 * IH], bf16,
                         kind="ExternalInput").ap()
    vp = nc.dram_tensor("vp", [PAIRS, JC, N_CHUNKS * (D + 1)], bf16,
                        kind="ExternalInput").ap()
    dl = nc.dram_tensor("dl", [PAIRS, JC, N_CHUNKS], f32,
                        kind="ExternalInput").ap()
    trim = nc.dram_tensor("trim", [JC, JC], bf16, kind="ExternalInput").ap()
    o = nc.dram_tensor("o", [PAIRS, D + 1, L], f32, kind="ExternalOutput").ap()

    with tile.TileContext(nc) as tc:
        with (
            tc.tile_pool(name="const", bufs=1) as const_pool,
            tc.tile_pool(name="qk0", bufs=2) as q0_pool,
            tc.tile_pool(name="qka", bufs=2) as qa_pool,
            tc.tile_pool(name="qkb", bufs=2) as qb_pool,
            tc.tile_pool(name="vpool", bufs=2) as v_pool,
            tc.tile_pool(name="dpool", bufs=2) as d_pool,
            tc.tile_pool(name="et", bufs=3) as et_pool,
            tc.tile_pool(name="ot", bufs=2) as ot_pool,
            tc.tile_pool(name="ps", bufs=2, space="PSUM") as ps_pool,
            tc.tile_pool(name="po", bufs=2, space="PSUM") as po_pool,
        ):
            trim_t = const_pool.tile([JC, JC], bf16, name="trim_t")
            nc.sync.dma_start(out=trim_t[:], in_=trim[:])

            for p in range(PAIRS):
                q0_t = q0_pool.tile([JC, 512 + JC], bf16, tag="qk0",
                                    name=f"q0{p}")
                qa_t = qa_pool.tile([JC, 2 * IH], bf16, tag="qka",
                                    name=f"qa{p}")
                qb_t = qb_pool.tile([JC, 2 * IH], bf16, tag="qkb",
                                    name=f"qb{p}")
                vp_t = v_pool.tile([JC, N_CHUNKS * (D + 1)], bf16, tag="vp",
                                   name=f"vp{p}")
                dl_t = d_pool.tile([JC, N_CHUNKS], f32, tag="dl",
                                   name=f"dl{p}")
                nc.sync.dma_start(out=q0_t[:], in_=qk0[p])
                nc.sync.dma_start(out=dl_t[:], in_=dl[p])
                nc.sync.dma_start(out=qa_t[:], in_=qkA[p])
                nc.sync.dma_start(out=vp_t[:], in_=vp[p])
                nc.sync.dma_start(out=qb_t[:], in_=qkB[p])

                def qt_cols(lo, hi):
                    # qt columns [lo:hi) of the logical [64, 2048] qt
                    assert (lo // IH) == ((hi - 1) // IH)
                    t = qa_t if lo < IH else qb_t
                    return t[:, lo % IH:(lo % IH) + (hi - lo)]

                def kt_cols(lo, hi):
                    t = qa_t if lo < IH else qb_t
                    return t[:, IH + lo % IH:IH + lo % IH + (hi - lo)]

                for half in range(L // IH):
                    i_lo = half * IH
                    nh = (i_lo + IH) // JC   # chunks this half
                    # last chunk contributing to each 512-col PSUM bank of po
                    # (the accumulation-group stop flag is bank-granular)
                    lastc = {0: (i_lo + 512) // JC - 1, 512: nh - 1}
                    po_t = po_pool.tile([D + 1, IH], f32, tag="po",
                                        name=f"po{p}_{half}")
                    ot_t = ot_pool.tile([D + 1, IH], f32, tag="ot",
                                        name=f"ot{p}_{half}")
                    for c in range(nh):
                        j0 = JC * c
                        x = max(0, j0 - i_lo)  # first valid i col in window
                        w = IH - x
                        fastpath = half == 0 and c == 0
                        splitexp = fastpath and p == 0
                        ps_t = ps_pool.tile([JC, IH], f32, tag="ps",
                                            name=f"ps{p}_{half}_{c}")
                        # matmul output is f32 and must stay in one PSUM bank
                        # -> split at 512 columns
                        for b0 in range(0, IH, 512):
                            lo = max(x, b0)
                            b1 = b0 + 512
                            if lo < b1:
                                if fastpath and b0 == 0:
                                    # chunk 0 bank 0 runs off the small qk0
                                    # tile (160 KB) before qkA lands
                                    kt_ap = q0_t[:, 0:JC]
                                    qt_ap = q0_t[:, JC:JC + 512]
                                else:
                                    kt_ap = kt_cols(j0, j0 + JC)
                                    qt_ap = qt_cols(i_lo + lo, i_lo + b1)
                                nc.tensor.matmul(
                                    ps_t[:, lo:b1], kt_ap, qt_ap,
                                    start=True, stop=True,
                                )
                        et_t = et_pool.tile([JC, IH], bf16, tag="et",
                                            name=f"et{p}_{half}_{c}")
                        if splitexp:
                            # split the first exp per bank so it can start
                            # while qkA is still streaming in
                            nc.scalar.activation(et_t[:, 0:512],
                                                 ps_t[:, 0:512], EXP,
                                                 bias=dl_t[:, c:c + 1])
                            nc.scalar.activation(et_t[:, 512:IH],
                                                 ps_t[:, 512:IH], EXP,
                                                 bias=dl_t[:, c:c + 1])
                        else:
                            nc.scalar.activation(et_t[:, 0:w], ps_t[:, x:IH],
                                                 EXP, bias=dl_t[:, c:c + 1])
                        if j0 >= i_lo:
                            # diagonal block: keep j <= i
                            nc.vector.tensor_mul(
                                et_t[:, 0:JC], et_t[:, 0:JC], trim_t[:])
                        # po += vp_c.T @ et, per PSUM bank (stop flags are
                        # bank-granular)
                        vch = vp_t[:, c * (D + 1):(c + 1) * (D + 1)]
                        first = c == 0
                        for b0 in range(0, IH, 512):
                            lo = max(x, b0)
                            b1 = b0 + 512
                            if lo < b1:
                                nc.tensor.matmul(
                                    po_t[:, lo:b1], vch,
                                    et_t[:, lo - x:b1 - x],
                                    start=first, stop=(c == lastc[b0]))
                        if c == lastc[0]:
                            # bank 0 of po is final: copy + store it while
                            # bank 1 keeps accumulating
                            nc.vector.tensor_copy(ot_t[:, 0:512],
                                                  po_t[:, 0:512])
                            nc.sync.dma_start(
                                out=o[p][:, i_lo:i_lo + 512],
                                in_=ot_t[:, 0:512])
                    nc.vector.tensor_copy(ot_t[:, 512:IH], po_t[:, 512:IH])
                    nc.sync.dma_start(out=o[p][:, i_lo + 512:i_lo + IH],
                                      in_=ot_t[:, 512:IH])

    nc.compile()
    _CACHED[key] = nc
    return nc


def _prep_core_inputs(queries, keys, values, tau, delta, core):
    bf = ml_dtypes.bfloat16
    qkA = np.zeros((PAIRS, JC, 2 * IH), dtype=bf)
    qkB = np.zeros((PAIRS, JC, 2 * IH), dtype=bf)
    vp = np.empty((PAIRS, JC, N_CHUNKS * (D + 1)), dtype=bf)
    dl = np.empty((PAIRS, JC, N_CHUNKS), dtype=np.float32)
    qk0 = np.zeros((PAIRS, JC, 512 + JC), dtype=bf)
    for p in range(PAIRS):
        g = core * PAIRS + p
        b, h = g // H, g % H
        qt = ((SCALE * tau[b, 0]) * queries[b, :, h, :].T).astype(bf)
        kt = keys[b, :, h, :].T.astype(bf)
        qk0[p, :E, :JC] = kt[:, :JC]
        qk0[p, :E, JC:] = qt[:, :512]
        qkA[p, :E, :IH] = qt[:, :IH]
        qkA[p, :E, IH:] = kt[:, :IH]
        qkB[p, :E, :IH] = qt[:, IH:]
        qkB[p, :E, IH:] = kt[:, IH:]
        v = values[b, :, h, :].reshape(N_CHUNKS, JC, D)
        vch = vp[p].reshape(JC, N_CHUNKS, D + 1)
        vch[:, :, :D] = v.transpose(1, 0, 2).astype(bf)
        vch[:, :, D] = 1.0
        dl[p] = (SCALE * delta[b, :]).reshape(N_CHUNKS, JC).T
    trim = np.triu(np.ones((JC, JC), dtype=bf))
    return {"qk0": qk0, "qkA": qkA, "qkB": qkB, "vp": vp, "dl": dl,
            "trim": trim}


def _run(queries, keys, values, tau, delta, trace=False, trace_kwargs=None):
    from concourse.bass_utils import run_bass_kernel_spmd

    queries = np.asarray(queries, dtype=np.float32)
    keys = np.asarray(keys, dtype=np.float32)
    values = np.asarray(values, dtype=np.float32)
    tau = np.asarray(tau, dtype=np.float32)
    delta = np.asarray(delta, dtype=np.float32)

    nc = _build_bass()
    in_maps = [
        _prep_core_inputs(queries, keys, values, tau, delta, core)
        for core in range(N_CORES)
    ]
    res = run_bass_kernel_spmd(
        nc, in_maps, list(range(N_CORES)), trace=trace,
        **(trace_kwargs or {}),
    )

    out = np.empty((B, L, H, D), dtype=np.float32)
    for core in range(N_CORES):
        o = res.results[core]["o"]  # [PAIRS, 65, L]
        for p in range(PAIRS):
            g = core * PAIRS + p
            b, h = g // H, g % H
            out[b, :, h, :] = (o[p, :D, :] / o[p, D:D + 1, :]).T
    return out, res


def kernel(queries, keys, values, tau, delta):
    out, _ = _run(queries, keys, values, tau, delta)
    return out
